# revision 8
# baseline (speedup 1.0000x reference)
"""KAN Convolutional Layer (3x3, Chebyshev degree 3, 8 convs) on 8 trn2 cores.

Math: the KAN conv's nonlinearities apply per input pixel (patches are shifted
copies of x), so the module reduces to 4 pointwise feature maps
    S = silu(x), T1 = tanh(x), T2 = 2*T1^2 - 1, T3 = (2*T2 - 1)*T1
convolved with a dense 3x3 kernel (4 feat channels -> 8 outputs per input
channel), plus a constant bias from T0 == 1. Zero-padding contributes 0 for
S/T1/T3 and -1 for T2: x-pads are materialized as columns (computed features of
0 give the right values automatically); y-pad contributions are folded into
per-row bias corrections.

On device each output 16-row block is one PSUM accumulation group of 13
fp16 matmuls: 1 bias (K=1 against a ones row) + 4 features x 3 dx-shifts
with banded K=128 weight matrices whose band encodes the y-offset, j, and tap
weights. M packs (j, y0_local) = 8*16 = 128; N packs (4 planes, 128 x) = 512.

End-to-end wall time is dominated by the axon tunnel (~25-80 MB/s with
~0.1s per-shard round-trip latency), so I/O is minimized and batched:
 - x ships as fp16 (8.4MB total);
 - all weights ship as ONE compact fp16 [19, 1536] block (row 0: bias row;
   rows 1-18: the banded taps, which depend only on r - y0l, not the group g)
   and are expanded on-device into the [128, 12288] banded layout by 8 DMAs;
 - the output is quantized on-device to int8 with a per-PSUM-row absmax
   scale (quantization error <= rowmax/127 < 0.8% of the global max); the
   scale itself is embedded in the output tensor (x-dim 128->130, last two
   int8 lanes hold the fp16 scale's bytes), so the kernel has exactly ONE
   output tensor to fetch and ONE donation buffer to upload.

Sharding: data-parallel over batch, 2 of 16 batch elements per core.
"""
import os

import numpy as np

N_CORES = 8
B_FULL, C, H, W = 16, 16, 128, 128
B_LOC = B_FULL // N_CORES          # 2 batch elements per core
NCONV = 8
PLANES_PER_GRP = 4                 # planes (b,c) batched into matmul N dim
N_GRP = B_LOC * C // PLANES_PER_GRP
WPAD = W + 2                       # x-padded width
WOUT = W + 2                       # output x-dim: 128 data + 2 scale bytes
BAND_R = 18                        # compact band rows: r = y - (16g - 1)

_CACHE = {}
LAST_RESULT = None


def _build_weights(cheby_coeffs, base_weight, spline_scaler):
    """One compact fp16 [19, 1536] block: row 0 bias vector, rows 1-18 band.

    band[r, (f*3+dx)*128 + j*16+y0l] = Wf[j, f, r-y0l, dx] for 0<=r-y0l<=2.
    The full banded lhsT for group g is the band placed at partitions
    16g-1+r (rows outside [0,128) dropped; their effect is folded into the
    bias rowfixes).
    """
    w = cheby_coeffs * spline_scaler[..., None]              # (8, 9, 4)
    Wf = np.stack([base_weight.reshape(8, 3, 3),             # f=0: silu
                   w[:, :, 1].reshape(8, 3, 3),              # f=1: T1
                   w[:, :, 2].reshape(8, 3, 3),              # f=2: T2
                   w[:, :, 3].reshape(8, 3, 3)], axis=1)     # (8,4,3,3) j,f,dy,dx
    bias = w[:, :, 0].sum(axis=1)                            # (8,)  T0 == 1
    rowfix_top = -w[:, 0:3, 2].sum(axis=1)                   # y=-1 pad, T2=-1
    rowfix_bot = -w[:, 6:9, 2].sum(axis=1)                   # y=128 pad

    r = np.arange(BAND_R)[:, None]                           # (18,1)
    j = (np.arange(128) // 16)[None, :]                      # (1,128)
    y0l = (np.arange(128) % 16)[None, :]
    dy = r - y0l
    valid = (dy >= 0) & (dy <= 2)
    dyc = np.clip(dy, 0, 2)
    wbc = np.zeros((1 + BAND_R, 12, 128), dtype=np.float32)
    for f in range(4):
        for dx in range(3):
            wbc[1:, f * 3 + dx, :] = np.where(valid, Wf[j, f, dyc, dx], 0.0)

    bv = np.empty((8, 128), dtype=np.float32)                # bias per (g -> m)
    jj, yl = np.arange(128) // 16, np.arange(128) % 16
    for g in range(8):
        v = bias[jj].copy()
        if g == 0:
            v[yl == 0] += rowfix_top[jj[yl == 0]]
        if g == 7:
            v[yl == 15] += rowfix_bot[jj[yl == 15]]
        bv[g] = v
    wbc[0, :8, :] = bv                                       # row 0, cols 0:1024
    return wbc.reshape(1 + BAND_R, 12 * 128).astype(np.float16)


def _build_nc():
    from concourse import bacc, mybir, tile

    f16, f32 = mybir.dt.float16, mybir.dt.float32
    i8 = mybir.dt.int8
    AF, ALU = mybir.ActivationFunctionType, mybir.AluOpType

    nc = bacc.Bacc("TRN2", target_bir_lowering=False)
    x_d = nc.dram_tensor("x", [B_LOC, C, H, W], f16, kind="ExternalInput")
    wbc_d = nc.dram_tensor("wbc", [1 + BAND_R, 1536], f16, kind="ExternalInput")
    o_d = nc.dram_tensor("o", [B_LOC, C * NCONV, H, WOUT], i8,
                         kind="ExternalOutput")

    with tile.TileContext(nc) as tc:
        with tc.tile_pool(name="wpool", bufs=1) as wpool, \
             tc.tile_pool(name="xpool", bufs=3) as xpool, \
             tc.tile_pool(name="fpool", bufs=2) as fpool, \
             tc.tile_pool(name="rpool", bufs=4) as rpool, \
             tc.tile_pool(name="opool", bufs=6) as opool, \
             tc.tile_pool(name="ppool", bufs=6, space="PSUM") as ppool:
            wb = wpool.tile([H, 8 * 1536], f16)
            bv = wpool.tile([1, 1024], f16)
            ones = wpool.tile([1, 512], f16)
            nc.vector.memset(wb[:], 0.0)
            nc.sync.dma_start(bv[:], wbc_d[0:1, 0:1024])
            nc.vector.memset(ones[:], 1.0)
            # expand compact band: group g occupies partitions 16g-1+r
            for g in range(8):
                wr0 = 1 if g == 0 else 0                 # drop y=-1 row
                nrow = BAND_R - wr0 - (1 if g == 7 else 0)  # drop y=128 row
                rlo = 16 * g - 1 + wr0
                nc.sync.dma_start(wb[rlo:rlo + nrow, g * 1536:(g + 1) * 1536],
                                  wbc_d[1 + wr0:1 + wr0 + nrow, :])

            for q in range(N_GRP):
                b, c0 = q // (C // PLANES_PER_GRP), PLANES_PER_GRP * (q % (C // PLANES_PER_GRP))
                xt = xpool.tile([H, PLANES_PER_GRP * WPAD], f16)
                xv = xt.rearrange("p (c x) -> p c x", c=PLANES_PER_GRP)
                nc.vector.memset(xv[:, :, 0:1], 0.0)
                nc.vector.memset(xv[:, :, WPAD - 1:WPAD], 0.0)
                nc.sync.dma_start(
                    xv[:, :, 1:W + 1],
                    x_d[b, c0:c0 + PLANES_PER_GRP].rearrange("c y x -> y c x"))

                S = fpool.tile([H, PLANES_PER_GRP * WPAD], f16)
                T1 = fpool.tile([H, PLANES_PER_GRP * WPAD], f16)
                T2 = fpool.tile([H, PLANES_PER_GRP * WPAD], f16)
                T3 = fpool.tile([H, PLANES_PER_GRP * WPAD], f16)
                nc.scalar.activation(S[:], xt[:], AF.Silu)
                nc.scalar.activation(T1[:], xt[:], AF.Tanh)
                nc.vector.tensor_mul(T2[:], T1[:], T1[:])
                nc.vector.tensor_scalar(T2[:], T2[:], 2.0, -1.0, ALU.mult, ALU.add)
                nc.vector.tensor_scalar(T3[:], T2[:], 2.0, -1.0, ALU.mult, ALU.add)
                nc.vector.tensor_mul(T3[:], T3[:], T1[:])
                feats = [S, T1, T2, T3]

                ov = o_d[b].rearrange("(c j) y x -> j y c x", j=NCONV)
                for g in range(8):
                    ps = ppool.tile([H, 512], f32)
                    nc.tensor.matmul(ps[:], bv[0:1, g * 128:(g + 1) * 128],
                                     ones[0:1, :], start=True, stop=False)
                    for f in range(4):
                        for dx in range(3):
                            lhsT = wb[:, (g * 12 + f * 3 + dx) * 128:
                                         (g * 12 + f * 3 + dx + 1) * 128]
                            rhs = feats[f].rearrange(
                                "p (c x) -> p c x", c=PLANES_PER_GRP)[:, :, dx:dx + W]
                            nc.tensor.matmul(
                                ps.rearrange("p (c x) -> p c x", c=PLANES_PER_GRP),
                                lhsT, rhs, start=False,
                                stop=(f == 3 and dx == 2))
                    # int8 quantization with per-PSUM-row absmax scale; the
                    # scale's fp16 bytes ride in the last two x-lanes.
                    rmax = rpool.tile([128, 1], f32)
                    rmax16 = rpool.tile([128, 1], f16)
                    rc = rpool.tile([128, 1], f32)
                    nc.vector.tensor_reduce(rmax[:], ps[:],
                                            axis=mybir.AxisListType.X,
                                            op=ALU.max, apply_absolute_value=True)
                    nc.vector.tensor_scalar_max(rmax[:], rmax[:], 1e-30)
                    nc.vector.tensor_copy(rmax16[:], rmax[:])
                    nc.vector.reciprocal(rc[:], rmax[:])
                    ot = opool.tile([H, PLANES_PER_GRP * WOUT], i8)
                    otv = ot.rearrange("p (c x) -> p c x", c=PLANES_PER_GRP)
                    nc.vector.tensor_scalar(otv[:, :, 0:W], ps.rearrange(
                        "p (c x) -> p c x", c=PLANES_PER_GRP), rc[:], 127.0,
                        ALU.mult, ALU.mult)
                    for ci in range(PLANES_PER_GRP):
                        nc.vector.tensor_copy(otv[:, ci, W:WOUT],
                                              rmax16[:].bitcast(i8))
                    # NOTE: DMA src APs must keep the partition dim unsplit
                    # (a split partition dim silently reads garbage), so one
                    # DMA per conv j with a contiguous 16-partition range.
                    for j in range(NCONV):
                        nc.sync.dma_start(
                            ov[j, 16 * g:16 * (g + 1), c0:c0 + PLANES_PER_GRP, :],
                            ot[j * 16:(j + 1) * 16, :].rearrange(
                                "p (c x) -> p c x", c=PLANES_PER_GRP))
    nc.finalize()
    return nc


def _enable_jax_compile_cache():
    """Persistent XLA compilation cache: the per-call re-lowering otherwise
    re-runs the neuron compile pipeline (~0.45s) on every invocation."""
    if _CACHE.get("jaxcc"):
        return
    try:
        import jax

        jax.config.update("jax_compilation_cache_dir", "/tmp/jaxcc")
        jax.config.update("jax_persistent_cache_min_entry_size_bytes", -1)
        jax.config.update("jax_persistent_cache_min_compile_time_secs", 0)
        # executables only — the extra XLA caches AOT-pin CPU executables
        # to compile-machine features, which warns/risks SIGILL on reload
        jax.config.update("jax_persistent_cache_enable_xla_caches", "none")
    except Exception:
        pass
    _CACHE["jaxcc"] = True


def kernel(x, cheby_coeffs, base_weight, spline_scaler):
    global LAST_RESULT
    from concourse.bass_utils import run_bass_kernel_spmd

    _enable_jax_compile_cache()
    x16 = np.asarray(x, dtype=np.float16)
    wbc = _build_weights(np.asarray(cheby_coeffs, np.float32),
                         np.asarray(base_weight, np.float32),
                         np.asarray(spline_scaler, np.float32))
    if "nc" not in _CACHE:
        _CACHE["nc"] = _build_nc()
    nc = _CACHE["nc"]

    in_maps = [{"x": x16[i * B_LOC:(i + 1) * B_LOC], "wbc": wbc}
               for i in range(N_CORES)]
    try:
        r = run_bass_kernel_spmd(nc, in_maps, core_ids=list(range(N_CORES)))
    except ModuleNotFoundError:
        # BASS_TRACE set but the axon NTFF profile hook isn't importable in
        # this container — rerun with tracing disabled.
        os.environ["BASS_NEVER_TRACE"] = "1"
        r = run_bass_kernel_spmd(nc, in_maps, core_ids=list(range(N_CORES)))
    except Exception:
        # Transient NRT_EXEC_UNIT_UNRECOVERABLE wedges have been observed
        # after heavy back-to-back runs; they clear after ~20s of idle.
        import time

        time.sleep(25.0)
        r = run_bass_kernel_spmd(nc, in_maps, core_ids=list(range(N_CORES)))
    LAST_RESULT = r

    out = np.empty((B_FULL, C * NCONV, H, W), np.float32)
    inv127 = np.float32(1.0 / 127.0)
    for i, res in enumerate(r.results):
        o2 = res["o"]                                  # (2,128,128,130) int8
        s = o2[..., W:WOUT].copy().view(np.float16).astype(np.float32)
        s *= inv127                                    # (2,128,128,1) scale
        np.multiply(o2[..., :W], s, out=out[i * B_LOC:(i + 1) * B_LOC],
                    casting="unsafe")
    return out
